# revision 9
# baseline (speedup 1.0000x reference)
"""Linear attention (elu(x)+1 feature map) Bass/Tile kernel for Trainium2.

Problem: B=4, H=16, S=4096, D=64, fp32.
  Qf = elu(Q)+1; Kf = (elu(K)+1)*mask
  KV = einsum('bhsd,bhse->bhde', Kf, V); Ksum = sum_s Kf
  out = (Qf @ KV) / (Qf . Ksum)

Sharding: the 64 (b,h) pairs are data-parallel; each of the 8 cores gets 8
pairs. No collectives.

Per-core design (v2 — DMA-dispatch-bound baseline restructured):
  * "Fat-row" layout: s = 32*p + r (partition p holds 32 consecutive rows).
    Each pair's Q/K/V/O then moves as ONE DMA of [128, 2048] with 8KB
    contiguous lines (vs 48 DMAs with 256B lines) — ~29 DMAs total.
  * bf16 matmul operands (tolerance is 2e-2): PE runs at 1 cycle/row.
  * elu(x)+1 = min(exp(x),1) + relu(x) in 3 passes: exp (ACT, bf16 out),
    in-place min (DVE), fused (x max 0) + e via scalar_tensor_tensor
    (Q on DVE, K on Pool).
  * mask folded into V during its bf16 conversion (V*m), and appended as
    column 64 of the vm tile so the KV accumulation matmul also yields
    Ksum = Kf^T m "for free" (merged A+B pairs: one [128,130]-wide matmul
    per 128-row step).
  * Qf^T via PE transposes batched 4-wide into one PSUM bank, copied to
    SBUF alternately by DVE/Pool.
  * Denominators Qf.Ksum computed densely per group via 32 tiny matmuls
    against block [KsumA|0; 0|KsumB], ONE reciprocal per group, then the
    PSUM->SBUF copy of the output matmul is fused with normalization
    (tensor_tensor multiply with stride-0 broadcast of rec).
  * Software-pipelined: group g's tail (bd/kc copies, den, phaseB,
    normalize, output DMA) issues during group g+1's head. Output DMAs go
    on the ACT queue to keep SP's input-DMA stream unblocked.
"""

import numpy as np

import concourse.bass as bass
import concourse.mybir as mybir
import concourse.tile as tile
from concourse.bass_utils import run_bass_kernel_spmd
from concourse.masks import make_identity

F32 = mybir.dt.float32
BF16 = mybir.dt.bfloat16
AF = mybir.ActivationFunctionType
ALU = mybir.AluOpType

N_CORES = 8
PAIRS = 8          # (b,h) pairs per core
S = 4096
D = 64
R = 32             # rows per partition; s = 32*p + r
NGROUPS = PAIRS // 2


def build_bass() -> bass.Bass:
    from contextlib import ExitStack

    from concourse.bacc import Bacc
    nc = Bacc()
    Qh = nc.dram_tensor("Q", [PAIRS, S, D], F32, kind="ExternalInput")
    Kh = nc.dram_tensor("K", [PAIRS, S, D], F32, kind="ExternalInput")
    Vh = nc.dram_tensor("V", [PAIRS, S, D], F32, kind="ExternalInput")
    Mh = nc.dram_tensor("mask", [PAIRS, S], F32, kind="ExternalInput")
    Oh = nc.dram_tensor("O", [PAIRS, S, D], F32, kind="ExternalOutput")

    # per-pair fat-row views [128, 2048]; per-group output views [128,2,2048]
    def pview(h, p):
        return h[p].rearrange("(q r) d -> q (r d)", q=128, r=R)

    Mv = Mh.rearrange("u (q r) -> q u r", q=128, r=R)          # [128, 8, 32]
    Ov = [Oh[2 * g:2 * g + 2].rearrange("u (q r) d -> q u (r d)", q=128, r=R)
          for g in range(NGROUPS)]

    with tile.TileContext(nc) as tc:
        with ExitStack() as ctx:
            consts = ctx.enter_context(tc.tile_pool(name="consts", bufs=1))
            qraw_p = ctx.enter_context(tc.tile_pool(name="qraw", bufs=2))
            kraw_p = ctx.enter_context(tc.tile_pool(name="kraw", bufs=2))
            vraw_p = ctx.enter_context(tc.tile_pool(name="vraw", bufs=2))
            qe_p = ctx.enter_context(tc.tile_pool(name="qe", bufs=2))
            ke_p = ctx.enter_context(tc.tile_pool(name="ke", bufs=2))
            qr2_p = ctx.enter_context(tc.tile_pool(name="qr2", bufs=2))
            kr2_p = ctx.enter_context(tc.tile_pool(name="kr2", bufs=2))
            qf_p = ctx.enter_context(tc.tile_pool(name="qf", bufs=2))
            kf_p = ctx.enter_context(tc.tile_pool(name="kf", bufs=2))
            vm_p = ctx.enter_context(tc.tile_pool(name="vm", bufs=2))
            qt_p = ctx.enter_context(tc.tile_pool(name="qt", bufs=2))
            osb_p = ctx.enter_context(tc.tile_pool(name="osb", bufs=2))
            rec_p = ctx.enter_context(tc.tile_pool(name="rec", bufs=2))
            bd_p = ctx.enter_context(tc.tile_pool(name="bd", bufs=2))
            kc_p = ctx.enter_context(tc.tile_pool(name="kc", bufs=2))
            kv_ps = ctx.enter_context(
                tc.tile_pool(name="kvps", bufs=2, space="PSUM"))
            tp_ps = ctx.enter_context(
                tc.tile_pool(name="tpps", bufs=2, space="PSUM"))
            den_ps = ctx.enter_context(
                tc.tile_pool(name="denps", bufs=2, space="PSUM"))
            ob_ps = ctx.enter_context(
                tc.tile_pool(name="obps", bufs=2, space="PSUM"))

            identity = consts.tile([128, 128], BF16)
            make_identity(nc, identity)
            mtile = consts.tile([128, PAIRS, R], F32)
            nc.sync.dma_start(out=mtile, in_=Mv)

            def tail(g, kv, qt):
                """den/recip/phaseB/normalize/output for a finished group."""
                bd = bd_p.tile([128, 2, D], BF16)
                nc.vector.memset(bd, 0.0)
                nc.vector.tensor_copy(out=bd[0:64, 0, :], in_=kv[0:64, 0, 0:D])
                nc.vector.tensor_copy(out=bd[64:128, 1, :],
                                      in_=kv[64:128, 1, 0:D])
                kc = kc_p.tile([128, 2], BF16)
                nc.vector.memset(kc, 0.0)
                nc.vector.tensor_copy(out=kc[0:64, 0:1], in_=kv[0:64, 0, D:D + 1])
                nc.vector.tensor_copy(out=kc[64:128, 1:2],
                                      in_=kv[64:128, 1, D:D + 1])

                den = den_ps.tile([128, R, 2], F32)
                for r in range(R):
                    nc.tensor.matmul(den[:, r, :], lhsT=qt[:, r, :], rhs=kc,
                                     start=True, stop=True)
                rec = rec_p.tile([128, R, 2], F32)
                nc.vector.reciprocal(rec, den)

                osb = osb_p.tile([128, 2, R, D], F32)
                for b in range(R // 4):
                    ob = ob_ps.tile([128, 4, 2, D], F32)
                    for j in range(4):
                        nc.tensor.matmul(ob[:, j], lhsT=qt[:, 4 * b + j, :],
                                         rhs=bd, start=True, stop=True)
                    nc.vector.tensor_tensor(
                        out=osb[:, :, 4 * b:4 * b + 4, :],
                        in0=ob.rearrange("q j u d -> q u j d"),
                        in1=rec[:, 4 * b:4 * b + 4, :]
                        .rearrange("q j u -> q u j").unsqueeze(-1)
                        .to_broadcast([128, 2, 4, D]),
                        op=ALU.mult)
                nc.scalar.dma_start(
                    out=Ov[g], in_=osb.rearrange("q u r d -> q u (r d)"))

            prev = None
            for p in range(PAIRS):
                g, u = divmod(p, 2)
                if u == 0:
                    kv = kv_ps.tile([128, 2, D + 1], F32)
                    qt = qt_p.tile([128, R, 128], BF16)
                    qf = qf_p.tile([128, R, 2, D], BF16)
                    kf = kf_p.tile([128, R, 2, D], BF16)
                    vm = vm_p.tile([128, R, 2, D + 1], BF16)

                qraw = qraw_p.tile([128, R * D], F32)
                kraw = kraw_p.tile([128, R * D], F32)
                vraw = vraw_p.tile([128, R * D], F32)
                nc.sync.dma_start(out=qraw, in_=pview(Qh, p))
                nc.sync.dma_start(out=kraw, in_=pview(Kh, p))
                nc.sync.dma_start(out=vraw, in_=pview(Vh, p))

                qrv = qraw.rearrange("q (r d) -> q r d", r=R)
                krv = kraw.rearrange("q (r d) -> q r d", r=R)
                vrv = vraw.rearrange("q (r d) -> q r d", r=R)

                qe = qe_p.tile([128, R * D], BF16)
                ke = ke_p.tile([128, R * D], BF16)
                qr2 = qr2_p.tile([128, R * D], BF16)
                kr2 = kr2_p.tile([128, R * D], BF16)
                qev = qe.rearrange("q (r d) -> q r d", r=R)
                kev = ke.rearrange("q (r d) -> q r d", r=R)
                qr2v = qr2.rearrange("q (r d) -> q r d", r=R)
                kr2v = kr2.rearrange("q (r d) -> q r d", r=R)

                # elu(x)+1 == min(exp(x),1) + relu(x)
                nc.scalar.activation(qe, qraw, AF.Exp)
                nc.scalar.activation(qr2, qraw, AF.Relu)
                nc.vector.scalar_tensor_tensor(
                    out=qf[:, :, u, :], in0=qev, scalar=1.0, in1=qr2v,
                    op0=ALU.min, op1=ALU.add)
                nc.scalar.activation(ke, kraw, AF.Exp)
                nc.scalar.activation(kr2, kraw, AF.Relu)
                nc.vector.scalar_tensor_tensor(
                    out=kf[:, :, u, :], in0=kev, scalar=1.0, in1=kr2v,
                    op0=ALU.min, op1=ALU.add)
                # vm[:,:,u,0:D] = V * mask (bf16), col D = mask
                nc.gpsimd.tensor_tensor(
                    out=vm[:, :, u, 0:D], in0=vrv,
                    in1=mtile[:, p, :].unsqueeze(-1).to_broadcast([128, R, D]),
                    op=ALU.mult)
                nc.gpsimd.tensor_copy(out=vm[:, :, u, D], in_=mtile[:, p, :])

                if u == 1:
                    # KV+Ksum accumulation: [128,130]-wide, 32 steps
                    for r in range(R):
                        nc.tensor.matmul(kv, lhsT=kf[:, r], rhs=vm[:, r],
                                         start=(r == 0), stop=(r == R - 1))
                    # Qf^T batched 4-wide
                    for b in range(R // 4):
                        tp = tp_ps.tile([128, 4, 128], BF16)
                        for j in range(4):
                            nc.tensor.transpose(tp[:, j], qf[:, 4 * b + j],
                                                identity)
                        nc.vector.tensor_copy(
                            out=qt[:, 4 * b:4 * b + 4, :], in_=tp)

                    if prev is not None:
                        tail(*prev)
                    prev = (g, kv, qt)
            tail(*prev)
    nc.finalize()
    return nc


_NC_CACHE = None


def _get_nc():
    global _NC_CACHE
    if _NC_CACHE is None:
        _NC_CACHE = build_bass()
    return _NC_CACHE


def kernel(Q: np.ndarray, K: np.ndarray, V: np.ndarray, mask: np.ndarray,
           _trace: bool = False):
    B, H = 4, 16
    NP = B * H
    per = NP // N_CORES
    Qr = np.ascontiguousarray(np.asarray(Q, dtype=np.float32).reshape(NP, S, D))
    Kr = np.ascontiguousarray(np.asarray(K, dtype=np.float32).reshape(NP, S, D))
    Vr = np.ascontiguousarray(np.asarray(V, dtype=np.float32).reshape(NP, S, D))
    Mr = np.ascontiguousarray(np.asarray(mask, dtype=np.float32).reshape(NP, S))

    in_maps = []
    for i in range(N_CORES):
        sl = slice(i * per, (i + 1) * per)
        in_maps.append({
            "Q": np.ascontiguousarray(Qr[sl]),
            "K": np.ascontiguousarray(Kr[sl]),
            "V": np.ascontiguousarray(Vr[sl]),
            "mask": np.ascontiguousarray(Mr[sl]),
        })

    nc = _get_nc()
    res = run_bass_kernel_spmd(nc, in_maps, core_ids=list(range(N_CORES)),
                               trace=_trace)
    out = np.concatenate([r["O"] for r in res.results], axis=0)
    if _trace:
        kernel._last_results = res
    return out.reshape(B, H, S, D)


# revision 10
# speedup vs baseline: 9.1370x; 9.1370x over previous
"""Linear attention (elu(x)+1 feature map) Bass/Tile kernel for Trainium2.

Problem: B=4, H=16, S=4096, D=64, fp32.
  Qf = elu(Q)+1; Kf = (elu(K)+1)*mask
  KV = einsum('bhsd,bhse->bhde', Kf, V); Ksum = sum_s Kf
  out = (Qf @ KV) / (Qf . Ksum)

Sharding: the 64 (b,h) pairs are data-parallel; each of the 8 cores gets 8
pairs. No collectives.

Per-core design (v2 — DMA-dispatch-bound baseline restructured):
  * "Fat-row" layout: s = 32*p + r (partition p holds 32 consecutive rows).
    Each pair's Q/K/V/O then moves as ONE DMA of [128, 2048] with 8KB
    contiguous lines (vs 48 DMAs with 256B lines) — ~29 DMAs total.
  * bf16 matmul operands (tolerance is 2e-2): PE runs at 1 cycle/row.
  * elu(x)+1 = min(exp(x),1) + relu(x) in 3 passes: exp (ACT, bf16 out),
    in-place min (DVE), fused (x max 0) + e via scalar_tensor_tensor
    (Q on DVE, K on Pool).
  * mask folded into V during its bf16 conversion (V*m), and appended as
    column 64 of the vm tile so the KV accumulation matmul also yields
    Ksum = Kf^T m "for free" (merged A+B pairs: one [128,130]-wide matmul
    per 128-row step).
  * Qf^T via PE transposes batched 4-wide into one PSUM bank, copied to
    SBUF alternately by DVE/Pool.
  * Denominators Qf.Ksum computed densely per group via 32 tiny matmuls
    against block [KsumA|0; 0|KsumB], ONE reciprocal per group, then the
    PSUM->SBUF copy of the output matmul is fused with normalization
    (tensor_tensor multiply with stride-0 broadcast of rec).
  * Software-pipelined: group g's tail (bd/kc copies, den, phaseB,
    normalize, output DMA) issues during group g+1's head. Output DMAs go
    on the ACT queue to keep SP's input-DMA stream unblocked.
"""

import numpy as np

import concourse.bass as bass
import concourse.mybir as mybir
import concourse.tile as tile
from concourse.bass_utils import run_bass_kernel_spmd
from concourse.masks import make_identity

F32 = mybir.dt.float32
BF16 = mybir.dt.bfloat16
AF = mybir.ActivationFunctionType
ALU = mybir.AluOpType

N_CORES = 8
PAIRS = 8          # (b,h) pairs per core
S = 4096
D = 64
R = 32             # rows per partition; s = 32*p + r
NGROUPS = PAIRS // 2


def build_bass(reps: int = 1) -> bass.Bass:
    from contextlib import ExitStack

    from concourse.bacc import Bacc
    nc = Bacc()
    Qh = nc.dram_tensor("Q", [PAIRS, S, D], F32, kind="ExternalInput")
    Kh = nc.dram_tensor("K", [PAIRS, S, D], F32, kind="ExternalInput")
    Vh = nc.dram_tensor("V", [PAIRS, S, D], F32, kind="ExternalInput")
    Mh = nc.dram_tensor("mask", [PAIRS, S], F32, kind="ExternalInput")
    Oh = nc.dram_tensor("O", [PAIRS, S, D], F32, kind="ExternalOutput")

    # per-pair fat-row views [128, 2048]; per-group output views [128,2,2048]
    def pview(h, p):
        return h[p].rearrange("(q r) d -> q (r d)", q=128, r=R)

    Mv = Mh.rearrange("u (q r) -> q u r", q=128, r=R)          # [128, 8, 32]
    Ov = [Oh[2 * g:2 * g + 2].rearrange("u (q r) d -> q u (r d)", q=128, r=R)
          for g in range(NGROUPS)]

    with tile.TileContext(nc) as tc:
        with ExitStack() as ctx:
            consts = ctx.enter_context(tc.tile_pool(name="consts", bufs=1))
            qraw_p = ctx.enter_context(tc.tile_pool(name="qraw", bufs=2))
            kraw_p = ctx.enter_context(tc.tile_pool(name="kraw", bufs=2))
            vraw_p = ctx.enter_context(tc.tile_pool(name="vraw", bufs=2))
            qe_p = ctx.enter_context(tc.tile_pool(name="qe", bufs=2))
            ke_p = ctx.enter_context(tc.tile_pool(name="ke", bufs=2))
            qr2_p = ctx.enter_context(tc.tile_pool(name="qr2", bufs=2))
            kr2_p = ctx.enter_context(tc.tile_pool(name="kr2", bufs=2))
            qf_p = ctx.enter_context(tc.tile_pool(name="qf", bufs=2))
            kf_p = ctx.enter_context(tc.tile_pool(name="kf", bufs=2))
            vm_p = ctx.enter_context(tc.tile_pool(name="vm", bufs=2))
            qt_p = ctx.enter_context(tc.tile_pool(name="qt", bufs=2))
            osb_p = ctx.enter_context(tc.tile_pool(name="osb", bufs=2))
            rec_p = ctx.enter_context(tc.tile_pool(name="rec", bufs=2))
            bd_p = ctx.enter_context(tc.tile_pool(name="bd", bufs=2))
            kc_p = ctx.enter_context(tc.tile_pool(name="kc", bufs=2))
            kv_ps = ctx.enter_context(
                tc.tile_pool(name="kvps", bufs=2, space="PSUM"))
            tp_ps = ctx.enter_context(
                tc.tile_pool(name="tpps", bufs=2, space="PSUM"))
            den_ps = ctx.enter_context(
                tc.tile_pool(name="denps", bufs=2, space="PSUM"))
            ob_ps = ctx.enter_context(
                tc.tile_pool(name="obps", bufs=2, space="PSUM"))

            identity = consts.tile([128, 128], BF16)
            make_identity(nc, identity)
            mtile = consts.tile([128, PAIRS, R], F32)
            nc.sync.dma_start(out=mtile, in_=Mv)

            def tail(g, kv, qt):
                g = g % NGROUPS
                """den/recip/phaseB/normalize/output for a finished group."""
                bd = bd_p.tile([128, 2, D], BF16)
                nc.vector.memset(bd, 0.0)
                nc.vector.tensor_copy(out=bd[0:64, 0, :], in_=kv[0:64, 0, 0:D])
                nc.vector.tensor_copy(out=bd[64:128, 1, :],
                                      in_=kv[64:128, 1, 0:D])
                kc = kc_p.tile([128, 2], BF16)
                nc.vector.memset(kc, 0.0)
                nc.vector.tensor_copy(out=kc[0:64, 0:1], in_=kv[0:64, 0, D:D + 1])
                nc.vector.tensor_copy(out=kc[64:128, 1:2],
                                      in_=kv[64:128, 1, D:D + 1])

                den = den_ps.tile([128, R, 2], F32)
                for r in range(R):
                    nc.tensor.matmul(den[:, r, :], lhsT=qt[:, r, :], rhs=kc,
                                     start=True, stop=True)
                rec = rec_p.tile([128, R, 2], F32)
                nc.vector.reciprocal(rec, den)

                osb = osb_p.tile([128, 2, R, D], F32)
                for b in range(R // 4):
                    ob = ob_ps.tile([128, 4, 2, D], F32)
                    for j in range(4):
                        nc.tensor.matmul(ob[:, j], lhsT=qt[:, 4 * b + j, :],
                                         rhs=bd, start=True, stop=True)
                    nc.vector.tensor_tensor(
                        out=osb[:, :, 4 * b:4 * b + 4, :],
                        in0=ob.rearrange("q j u d -> q u j d"),
                        in1=rec[:, 4 * b:4 * b + 4, :]
                        .rearrange("q j u -> q u j").unsqueeze(-1)
                        .to_broadcast([128, 2, 4, D]),
                        op=ALU.mult)
                nc.scalar.dma_start(
                    out=Ov[g], in_=osb.rearrange("q u r d -> q u (r d)"))

            prev = None
            for p in range(PAIRS * reps):
                g, u = divmod(p % PAIRS, 2)
                g += (p // PAIRS) * NGROUPS
                p = p % PAIRS
                if u == 0:
                    kv = kv_ps.tile([128, 2, D + 1], F32)
                    qt = qt_p.tile([128, R, 128], BF16)
                    qf = qf_p.tile([128, R, 2, D], BF16)
                    kf = kf_p.tile([128, R, 2, D], BF16)
                    vm = vm_p.tile([128, R, 2, D + 1], BF16)

                qraw = qraw_p.tile([128, R * D], F32)
                kraw = kraw_p.tile([128, R * D], F32)
                vraw = vraw_p.tile([128, R * D], F32)
                nc.sync.dma_start(out=qraw, in_=pview(Qh, p))
                nc.sync.dma_start(out=kraw, in_=pview(Kh, p))
                nc.sync.dma_start(out=vraw, in_=pview(Vh, p))

                qrv = qraw.rearrange("q (r d) -> q r d", r=R)
                krv = kraw.rearrange("q (r d) -> q r d", r=R)
                vrv = vraw.rearrange("q (r d) -> q r d", r=R)

                qe = qe_p.tile([128, R * D], BF16)
                ke = ke_p.tile([128, R * D], BF16)
                qr2 = qr2_p.tile([128, R * D], BF16)
                kr2 = kr2_p.tile([128, R * D], BF16)
                qev = qe.rearrange("q (r d) -> q r d", r=R)
                kev = ke.rearrange("q (r d) -> q r d", r=R)
                qr2v = qr2.rearrange("q (r d) -> q r d", r=R)
                kr2v = kr2.rearrange("q (r d) -> q r d", r=R)

                # elu(x)+1 == min(exp(x),1) + relu(x)
                nc.scalar.activation(qe, qraw, AF.Exp)
                nc.scalar.activation(qr2, qraw, AF.Relu)
                nc.vector.scalar_tensor_tensor(
                    out=qf[:, :, u, :], in0=qev, scalar=1.0, in1=qr2v,
                    op0=ALU.min, op1=ALU.add)
                nc.scalar.activation(ke, kraw, AF.Exp)
                nc.scalar.activation(kr2, kraw, AF.Relu)
                nc.vector.scalar_tensor_tensor(
                    out=kf[:, :, u, :], in0=kev, scalar=1.0, in1=kr2v,
                    op0=ALU.min, op1=ALU.add)
                # vm[:,:,u,0:D] = V * mask (bf16), col D = mask
                nc.gpsimd.tensor_tensor(
                    out=vm[:, :, u, 0:D], in0=vrv,
                    in1=mtile[:, p, :].unsqueeze(-1).to_broadcast([128, R, D]),
                    op=ALU.mult)
                nc.gpsimd.tensor_copy(out=vm[:, :, u, D], in_=mtile[:, p, :])

                if u == 1:
                    # KV+Ksum accumulation: [128,130]-wide, 32 steps
                    for r in range(R):
                        nc.tensor.matmul(kv, lhsT=kf[:, r], rhs=vm[:, r],
                                         start=(r == 0), stop=(r == R - 1))
                    # Qf^T batched 4-wide
                    for b in range(R // 4):
                        tp = tp_ps.tile([128, 4, 128], BF16)
                        for j in range(4):
                            nc.tensor.transpose(tp[:, j], qf[:, 4 * b + j],
                                                identity)
                        nc.vector.tensor_copy(
                            out=qt[:, 4 * b:4 * b + 4, :], in_=tp)

                    if prev is not None:
                        tail(*prev)
                    prev = (g, kv, qt)
            tail(*prev)
    nc.finalize()
    return nc


_NC_CACHE = None


def _get_nc():
    global _NC_CACHE
    if _NC_CACHE is None:
        _NC_CACHE = build_bass()
    return _NC_CACHE


def kernel(Q: np.ndarray, K: np.ndarray, V: np.ndarray, mask: np.ndarray,
           _trace: bool = False):
    B, H = 4, 16
    NP = B * H
    per = NP // N_CORES
    Qr = np.ascontiguousarray(np.asarray(Q, dtype=np.float32).reshape(NP, S, D))
    Kr = np.ascontiguousarray(np.asarray(K, dtype=np.float32).reshape(NP, S, D))
    Vr = np.ascontiguousarray(np.asarray(V, dtype=np.float32).reshape(NP, S, D))
    Mr = np.ascontiguousarray(np.asarray(mask, dtype=np.float32).reshape(NP, S))

    in_maps = []
    for i in range(N_CORES):
        sl = slice(i * per, (i + 1) * per)
        in_maps.append({
            "Q": np.ascontiguousarray(Qr[sl]),
            "K": np.ascontiguousarray(Kr[sl]),
            "V": np.ascontiguousarray(Vr[sl]),
            "mask": np.ascontiguousarray(Mr[sl]),
        })

    nc = _get_nc()
    res = run_bass_kernel_spmd(nc, in_maps, core_ids=list(range(N_CORES)),
                               trace=_trace)
    out = np.concatenate([r["O"] for r in res.results], axis=0)
    if _trace:
        kernel._last_results = res
    return out.reshape(B, H, S, D)


# revision 18
# speedup vs baseline: 13.0068x; 1.4235x over previous
"""Linear attention (elu(x)+1 feature map) Bass/Tile kernel for Trainium2.

Problem: B=4, H=16, S=4096, D=64, fp32.
  Qf = elu(Q)+1; Kf = (elu(K)+1)*mask
  KV = einsum('bhsd,bhse->bhde', Kf, V); Ksum = sum_s Kf
  out = (Qf @ KV) / (Qf . Ksum)

Sharding: the 64 (b,h) pairs are data-parallel; each of the 8 cores gets 8
pairs. No collectives.

Per-core design (v2 — DMA-dispatch-bound baseline restructured):
  * "Fat-row" layout: s = 32*p + r (partition p holds 32 consecutive rows).
    Each pair's Q/K/V/O then moves as ONE DMA of [128, 2048] with 8KB
    contiguous lines (vs 48 DMAs with 256B lines) — ~29 DMAs total.
  * bf16 matmul operands (tolerance is 2e-2): PE runs at 1 cycle/row.
  * elu(x)+1 = min(exp(x),1) + relu(x) in 3 passes: exp (ACT, bf16 out),
    in-place min (DVE), fused (x max 0) + e via scalar_tensor_tensor
    (Q on DVE, K on Pool).
  * mask folded into V during its bf16 conversion (V*m), and appended as
    column 64 of the vm tile so the KV accumulation matmul also yields
    Ksum = Kf^T m "for free" (merged A+B pairs: one [128,130]-wide matmul
    per 128-row step).
  * Qf^T via PE transposes batched 4-wide into one PSUM bank, copied to
    SBUF alternately by DVE/Pool.
  * Denominators Qf.Ksum computed densely per group via 32 tiny matmuls
    against block [KsumA|0; 0|KsumB], ONE reciprocal per group, then the
    PSUM->SBUF copy of the output matmul is fused with normalization
    (tensor_tensor multiply with stride-0 broadcast of rec).
  * Software-pipelined: group g's tail (bd/kc copies, den, phaseB,
    normalize, output DMA) issues during group g+1's head. Output DMAs go
    on the ACT queue to keep SP's input-DMA stream unblocked.
"""

import numpy as np

import concourse.bass as bass
import concourse.mybir as mybir
import concourse.tile as tile
from concourse.bass_utils import run_bass_kernel_spmd
from concourse.masks import make_identity

F32 = mybir.dt.float32
BF16 = mybir.dt.bfloat16
AF = mybir.ActivationFunctionType
ALU = mybir.AluOpType

N_CORES = 8
PAIRS = 8          # (b,h) pairs per core
S = 4096
D = 64
R = 32             # rows per partition; s = 32*p + r
NGROUPS = PAIRS // 2


def build_bass(reps: int = 1) -> bass.Bass:
    from contextlib import ExitStack

    from concourse.bacc import Bacc
    nc = Bacc()
    Qh = nc.dram_tensor("Q", [PAIRS, S, D], F32, kind="ExternalInput")
    Kh = nc.dram_tensor("K", [PAIRS, S, D], F32, kind="ExternalInput")
    Vh = nc.dram_tensor("V", [PAIRS, S, D], F32, kind="ExternalInput")
    Mh = nc.dram_tensor("mask", [PAIRS, S], F32, kind="ExternalInput")
    Oh = nc.dram_tensor("O", [PAIRS, S, D], F32, kind="ExternalOutput")

    # per-pair fat-row views [128, 2048]; per-group output views [128,2,2048]
    def pview(h, p):
        return h[p].rearrange("(q r) d -> q (r d)", q=128, r=R)

    Mv = Mh.rearrange("u (q r) -> q u r", q=128, r=R)          # [128, 8, 32]
    Ov = [Oh[2 * g:2 * g + 2].rearrange("u (q r) d -> q u (r d)", q=128, r=R)
          for g in range(NGROUPS)]

    with tile.TileContext(nc) as tc:
        with ExitStack() as ctx:
            consts = ctx.enter_context(tc.tile_pool(name="consts", bufs=1))
            qraw_p = ctx.enter_context(tc.tile_pool(name="qraw", bufs=2))
            kraw_p = ctx.enter_context(tc.tile_pool(name="kraw", bufs=2))
            vraw_p = ctx.enter_context(tc.tile_pool(name="vraw", bufs=2))
            qe_p = ctx.enter_context(tc.tile_pool(name="qe", bufs=2))
            ke_p = ctx.enter_context(tc.tile_pool(name="ke", bufs=2))
            qr2_p = ctx.enter_context(tc.tile_pool(name="qr2", bufs=2))
            kr2_p = ctx.enter_context(tc.tile_pool(name="kr2", bufs=2))
            qf_p = ctx.enter_context(tc.tile_pool(name="qf", bufs=2))
            kf_p = ctx.enter_context(tc.tile_pool(name="kf", bufs=2))
            vm_p = ctx.enter_context(tc.tile_pool(name="vm", bufs=2))
            qt_p = ctx.enter_context(tc.tile_pool(name="qt", bufs=2))
            osb_p = ctx.enter_context(tc.tile_pool(name="osb", bufs=2))
            bd_p = ctx.enter_context(tc.tile_pool(name="bd", bufs=2))
            rec_p = ctx.enter_context(tc.tile_pool(name="rec", bufs=2))
            kv_ps = ctx.enter_context(
                tc.tile_pool(name="kvps", bufs=2, space="PSUM"))
            tp_ps = ctx.enter_context(
                tc.tile_pool(name="tpps", bufs=2, space="PSUM"))
            ob_ps = ctx.enter_context(
                tc.tile_pool(name="obps", bufs=3, space="PSUM"))

            identity = consts.tile([128, 128], BF16)
            make_identity(nc, identity)
            mtile = consts.tile([128, PAIRS, R], F32)
            nc.sync.dma_start(out=mtile, in_=Mv)

            def tail(g, kv, qt, last=False):
                g = g % NGROUPS
                """phaseB (out + den cols) / recip / normalize / output."""
                # bd = [[KV_A|KsumA], 0; 0, [KV_B|KsumB]] — one copy per pair
                bd = bd_p.tile([128, 2, D + 1], BF16)
                nc.vector.memset(bd, 0.0)
                nc.vector.tensor_copy(out=bd[0:64, 0, :], in_=kv[0:64, 0, :])
                nc.vector.tensor_copy(out=bd[64:128, 1, :],
                                      in_=kv[64:128, 1, :])

                osb = osb_p.tile([128, 2, R, D], F32)
                rec = rec_p.tile([128, R, 2], F32)
                # 32 r-steps in batches of 3 per PSUM bank (3*130 <= 512 f32)
                batches = [(s, min(3, R - s)) for s in range(0, R, 3)]
                # output DMA chunk boundaries (batch idx -> r range end)
                cuts = [4, 8] if last else [6]
                prev_r = [0]
                for bi, (r0, bsz) in enumerate(batches):
                    ob = ob_ps.tile([128, 3, 2, D + 1], F32)
                    for j in range(bsz):
                        nc.tensor.matmul(ob[:, j], lhsT=qt[:, r0 + j, :],
                                         rhs=bd, start=True, stop=True)
                    nc.vector.reciprocal(
                        rec[:, r0:r0 + bsz, :], ob[:, 0:bsz, :, D])
                    nc.vector.tensor_tensor(
                        out=osb[:, :, r0:r0 + bsz, :],
                        in0=ob[:, 0:bsz, :, 0:D].rearrange(
                            "q j u d -> q u j d"),
                        in1=rec[:, r0:r0 + bsz, :]
                        .rearrange("q j u -> q u j").unsqueeze(-1)
                        .to_broadcast([128, 2, bsz, D]),
                        op=ALU.mult)
                    if bi + 1 in cuts:
                        a, b2 = prev_r[0], r0 + bsz
                        nc.scalar.dma_start(
                            out=Ov[g][:, :, a * D:b2 * D],
                            in_=osb[:, :, a:b2, :].rearrange(
                                "q u r d -> q u (r d)"))
                        prev_r[0] = b2
                a = prev_r[0]
                nc.scalar.dma_start(
                    out=Ov[g][:, :, a * D:],
                    in_=osb[:, :, a:, :].rearrange("q u r d -> q u (r d)"))

            prev = None
            for p in range(PAIRS * reps):
                g, u = divmod(p % PAIRS, 2)
                g += (p // PAIRS) * NGROUPS
                p = p % PAIRS
                if u == 0:
                    kv = kv_ps.tile([128, 2, D + 1], F32)
                    qt = qt_p.tile([128, R, 128], BF16)
                    qf = qf_p.tile([128, R, 2, D], BF16)
                    kf = kf_p.tile([128, R, 2, D], BF16)
                    vm = vm_p.tile([128, R, 2, D + 1], BF16)

                qraw = qraw_p.tile([128, R * D], F32)
                kraw = kraw_p.tile([128, R * D], F32)
                vraw = vraw_p.tile([128, R * D], F32)
                nc.sync.dma_start(out=qraw, in_=pview(Qh, p))
                nc.sync.dma_start(out=kraw, in_=pview(Kh, p))
                nc.sync.dma_start(out=vraw, in_=pview(Vh, p))

                qrv = qraw.rearrange("q (r d) -> q r d", r=R)
                krv = kraw.rearrange("q (r d) -> q r d", r=R)
                vrv = vraw.rearrange("q (r d) -> q r d", r=R)

                qe = qe_p.tile([128, R * D], BF16)
                ke = ke_p.tile([128, R * D], BF16)
                qr2 = qr2_p.tile([128, R * D], BF16)
                kr2 = kr2_p.tile([128, R * D], BF16)
                qev = qe.rearrange("q (r d) -> q r d", r=R)
                kev = ke.rearrange("q (r d) -> q r d", r=R)
                qr2v = qr2.rearrange("q (r d) -> q r d", r=R)
                kr2v = kr2.rearrange("q (r d) -> q r d", r=R)

                # elu(x)+1 == min(exp(x),1) + relu(x)
                nc.scalar.activation(qe, qraw, AF.Exp)
                nc.scalar.activation(qr2, qraw, AF.Relu)
                nc.vector.scalar_tensor_tensor(
                    out=qf[:, :, u, :], in0=qev, scalar=1.0, in1=qr2v,
                    op0=ALU.min, op1=ALU.add)
                nc.scalar.activation(ke, kraw, AF.Exp)
                nc.gpsimd.tensor_scalar_max(kr2, kraw, 0.0)
                nc.vector.scalar_tensor_tensor(
                    out=kf[:, :, u, :], in0=kev, scalar=1.0, in1=kr2v,
                    op0=ALU.min, op1=ALU.add)
                # vm[:,:,u,0:D] = V * mask (bf16), col D = mask
                nc.gpsimd.tensor_tensor(
                    out=vm[:, :, u, 0:D], in0=vrv,
                    in1=mtile[:, p, :].unsqueeze(-1).to_broadcast([128, R, D]),
                    op=ALU.mult)
                nc.gpsimd.tensor_copy(out=vm[:, :, u, D], in_=mtile[:, p, :])

                if u == 1:
                    if prev is not None:
                        tail(*prev)
                        prev = None
                    # KV+Ksum accumulation: [128,130]-wide, 32 steps
                    for r in range(R):
                        nc.tensor.matmul(kv, lhsT=kf[:, r], rhs=vm[:, r],
                                         start=(r == 0), stop=(r == R - 1))
                    # Qf^T batched 4-wide
                    for b in range(R // 4):
                        tp = tp_ps.tile([128, 4, 128], BF16)
                        for j in range(4):
                            nc.tensor.transpose(tp[:, j], qf[:, 4 * b + j],
                                                identity)
                        nc.scalar.activation(
                            qt[:, 4 * b:4 * b + 4, :], tp, AF.Copy)

                    prev = (g, kv, qt)
            tail(*prev, last=True)
    nc.finalize()
    return nc


_NC_CACHE = None


def _get_nc():
    global _NC_CACHE
    if _NC_CACHE is None:
        _NC_CACHE = build_bass()
    return _NC_CACHE


def kernel(Q: np.ndarray, K: np.ndarray, V: np.ndarray, mask: np.ndarray,
           _trace: bool = False):
    B, H = 4, 16
    NP = B * H
    per = NP // N_CORES
    Qr = np.ascontiguousarray(np.asarray(Q, dtype=np.float32).reshape(NP, S, D))
    Kr = np.ascontiguousarray(np.asarray(K, dtype=np.float32).reshape(NP, S, D))
    Vr = np.ascontiguousarray(np.asarray(V, dtype=np.float32).reshape(NP, S, D))
    Mr = np.ascontiguousarray(np.asarray(mask, dtype=np.float32).reshape(NP, S))

    in_maps = []
    for i in range(N_CORES):
        sl = slice(i * per, (i + 1) * per)
        in_maps.append({
            "Q": np.ascontiguousarray(Qr[sl]),
            "K": np.ascontiguousarray(Kr[sl]),
            "V": np.ascontiguousarray(Vr[sl]),
            "mask": np.ascontiguousarray(Mr[sl]),
        })

    nc = _get_nc()
    res = run_bass_kernel_spmd(nc, in_maps, core_ids=list(range(N_CORES)),
                               trace=_trace)
    out = np.concatenate([r["O"] for r in res.results], axis=0)
    if _trace:
        kernel._last_results = res
    return out.reshape(B, H, S, D)


# revision 19
# speedup vs baseline: 36.9643x; 2.8419x over previous
"""Linear attention (elu(x)+1 feature map) Bass/Tile kernel for Trainium2.

Problem: B=4, H=16, S=4096, D=64, fp32.
  Qf = elu(Q)+1; Kf = (elu(K)+1)*mask
  KV = einsum('bhsd,bhse->bhde', Kf, V); Ksum = sum_s Kf
  out = (Qf @ KV) / (Qf . Ksum)

Sharding: the 64 (b,h) pairs are data-parallel; each of the 8 cores gets 8
pairs. No collectives.

Per-core design (v2 — DMA-dispatch-bound baseline restructured):
  * "Fat-row" layout: s = 32*p + r (partition p holds 32 consecutive rows).
    Each pair's Q/K/V/O then moves as ONE DMA of [128, 2048] with 8KB
    contiguous lines (vs 48 DMAs with 256B lines) — ~29 DMAs total.
  * bf16 matmul operands (tolerance is 2e-2): PE runs at 1 cycle/row.
  * elu(x)+1 = min(exp(x),1) + relu(x) in 3 passes: exp (ACT, bf16 out),
    in-place min (DVE), fused (x max 0) + e via scalar_tensor_tensor
    (Q on DVE, K on Pool).
  * mask folded into V during its bf16 conversion (V*m), and appended as
    column 64 of the vm tile so the KV accumulation matmul also yields
    Ksum = Kf^T m "for free" (merged A+B pairs: one [128,130]-wide matmul
    per 128-row step).
  * Qf^T via PE transposes batched 4-wide into one PSUM bank, copied to
    SBUF alternately by DVE/Pool.
  * Denominators Qf.Ksum computed densely per group via 32 tiny matmuls
    against block [KsumA|0; 0|KsumB], ONE reciprocal per group, then the
    PSUM->SBUF copy of the output matmul is fused with normalization
    (tensor_tensor multiply with stride-0 broadcast of rec).
  * Software-pipelined: group g's tail (bd/kc copies, den, phaseB,
    normalize, output DMA) issues during group g+1's head. Output DMAs go
    on the ACT queue to keep SP's input-DMA stream unblocked.
"""

import numpy as np

import concourse.bass as bass
import concourse.mybir as mybir
import concourse.tile as tile
from concourse.bass_utils import run_bass_kernel_spmd
from concourse.masks import make_identity

F32 = mybir.dt.float32
BF16 = mybir.dt.bfloat16
AF = mybir.ActivationFunctionType
ALU = mybir.AluOpType

N_CORES = 8
PAIRS = 8          # (b,h) pairs per core
S = 4096
D = 64
R = 32             # rows per partition; s = 32*p + r
NGROUPS = PAIRS // 2


def build_bass(reps: int = 1) -> bass.Bass:
    from contextlib import ExitStack

    from concourse.bacc import Bacc
    nc = Bacc()
    Qh = nc.dram_tensor("Q", [PAIRS, S, D], F32, kind="ExternalInput")
    Kh = nc.dram_tensor("K", [PAIRS, S, D], F32, kind="ExternalInput")
    Vh = nc.dram_tensor("V", [PAIRS, S, D], F32, kind="ExternalInput")
    Mh = nc.dram_tensor("mask", [PAIRS, S], F32, kind="ExternalInput")
    Oh = nc.dram_tensor("O", [PAIRS, S, D], F32, kind="ExternalOutput")

    # per-pair fat-row views [128, 2048]; per-group output views [128,2,2048]
    def pview(h, p):
        return h[p].rearrange("(q r) d -> q (r d)", q=128, r=R)

    Mv = Mh.rearrange("u (q r) -> q u r", q=128, r=R)          # [128, 8, 32]
    Ov = [Oh[2 * g:2 * g + 2].rearrange("u (q r) d -> q u (r d)", q=128, r=R)
          for g in range(NGROUPS)]

    with tile.TileContext(nc) as tc:
        with ExitStack() as ctx:
            consts = ctx.enter_context(tc.tile_pool(name="consts", bufs=1))
            qraw_p = ctx.enter_context(tc.tile_pool(name="qraw", bufs=2))
            kraw_p = ctx.enter_context(tc.tile_pool(name="kraw", bufs=2))
            vraw_p = ctx.enter_context(tc.tile_pool(name="vraw", bufs=2))
            qe_p = ctx.enter_context(tc.tile_pool(name="qe", bufs=2))
            ke_p = ctx.enter_context(tc.tile_pool(name="ke", bufs=2))
            qr2_p = ctx.enter_context(tc.tile_pool(name="qr2", bufs=2))
            kr2_p = ctx.enter_context(tc.tile_pool(name="kr2", bufs=2))
            qf_p = ctx.enter_context(tc.tile_pool(name="qf", bufs=2))
            kf_p = ctx.enter_context(tc.tile_pool(name="kf", bufs=2))
            vm_p = ctx.enter_context(tc.tile_pool(name="vm", bufs=2))
            qt_p = ctx.enter_context(tc.tile_pool(name="qt", bufs=2))
            osb_p = ctx.enter_context(tc.tile_pool(name="osb", bufs=2))
            bd_p = ctx.enter_context(tc.tile_pool(name="bd", bufs=2))
            rec_p = ctx.enter_context(tc.tile_pool(name="rec", bufs=2))
            kv_ps = ctx.enter_context(
                tc.tile_pool(name="kvps", bufs=2, space="PSUM"))
            tp_ps = ctx.enter_context(
                tc.tile_pool(name="tpps", bufs=2, space="PSUM"))
            ob_ps = ctx.enter_context(
                tc.tile_pool(name="obps", bufs=3, space="PSUM"))

            identity = consts.tile([128, 128], BF16)
            make_identity(nc, identity)
            mtile = consts.tile([128, PAIRS, R], F32)
            nc.sync.dma_start(out=mtile, in_=Mv)

            def tail(g, kv, qt, last=False):
                g = g % NGROUPS
                """phaseB (out + den cols) / recip / normalize / output."""
                # bd = [[KV_A|KsumA], 0; 0, [KV_B|KsumB]] — one copy per pair
                bd = bd_p.tile([128, 2, D + 1], BF16)
                nc.vector.memset(bd, 0.0)
                nc.vector.tensor_copy(out=bd[0:64, 0, :], in_=kv[0:64, 0, :])
                nc.vector.tensor_copy(out=bd[64:128, 1, :],
                                      in_=kv[64:128, 1, :])

                osb = osb_p.tile([128, 2, R, D], F32)
                rec = rec_p.tile([128, R, 2], F32)
                # 32 r-steps in batches of 3 per PSUM bank (3*130 <= 512 f32)
                batches = [(s, min(3, R - s)) for s in range(0, R, 3)]
                # output DMA chunk boundaries (batch idx -> r range end)
                cuts = [4, 8] if last else [6]
                prev_r = [0]
                for bi, (r0, bsz) in enumerate(batches):
                    ob = ob_ps.tile([128, 3, 2, D + 1], F32)
                    for j in range(bsz):
                        nc.tensor.matmul(ob[:, j], lhsT=qt[:, r0 + j, :],
                                         rhs=bd, start=True, stop=True)
                    nc.vector.reciprocal(
                        rec[:, r0:r0 + bsz, :], ob[:, 0:bsz, :, D])
                    nc.vector.tensor_tensor(
                        out=osb[:, :, r0:r0 + bsz, :],
                        in0=ob[:, 0:bsz, :, 0:D].rearrange(
                            "q j u d -> q u j d"),
                        in1=rec[:, r0:r0 + bsz, :]
                        .rearrange("q j u -> q u j").unsqueeze(-1)
                        .to_broadcast([128, 2, bsz, D]),
                        op=ALU.mult)
                    if bi + 1 in cuts:
                        a, b2 = prev_r[0], r0 + bsz
                        nc.scalar.dma_start(
                            out=Ov[g][:, :, a * D:b2 * D],
                            in_=osb[:, :, a:b2, :].rearrange(
                                "q u r d -> q u (r d)"))
                        prev_r[0] = b2
                a = prev_r[0]
                nc.scalar.dma_start(
                    out=Ov[g][:, :, a * D:],
                    in_=osb[:, :, a:, :].rearrange("q u r d -> q u (r d)"))

            prev = None
            for p in range(PAIRS * reps):
                g, u = divmod(p % PAIRS, 2)
                g += (p // PAIRS) * NGROUPS
                p = p % PAIRS
                if u == 0:
                    kv = kv_ps.tile([128, 2, D + 1], F32)
                    qt = qt_p.tile([128, R, 128], BF16)
                    qf = qf_p.tile([128, R, 2, D], BF16)
                    kf = kf_p.tile([128, R, 2, D], BF16)
                    vm = vm_p.tile([128, R, 2, D + 1], BF16)

                qraw = qraw_p.tile([128, R * D], F32)
                kraw = kraw_p.tile([128, R * D], F32)
                vraw = vraw_p.tile([128, R * D], F32)
                nc.sync.dma_start(out=qraw, in_=pview(Qh, p))
                nc.sync.dma_start(out=kraw, in_=pview(Kh, p))
                nc.sync.dma_start(out=vraw, in_=pview(Vh, p))

                qrv = qraw.rearrange("q (r d) -> q r d", r=R)
                krv = kraw.rearrange("q (r d) -> q r d", r=R)
                vrv = vraw.rearrange("q (r d) -> q r d", r=R)

                qe = qe_p.tile([128, R * D], BF16)
                ke = ke_p.tile([128, R * D], BF16)
                qr2 = qr2_p.tile([128, R * D], BF16)
                kr2 = kr2_p.tile([128, R * D], BF16)
                qev = qe.rearrange("q (r d) -> q r d", r=R)
                kev = ke.rearrange("q (r d) -> q r d", r=R)
                qr2v = qr2.rearrange("q (r d) -> q r d", r=R)
                kr2v = kr2.rearrange("q (r d) -> q r d", r=R)

                # elu(x)+1 == min(exp(x),1) + relu(x)
                nc.scalar.activation(qe, qraw, AF.Exp)
                nc.scalar.activation(qr2, qraw, AF.Relu)
                nc.vector.scalar_tensor_tensor(
                    out=qf[:, :, u, :], in0=qev, scalar=1.0, in1=qr2v,
                    op0=ALU.min, op1=ALU.add)
                nc.scalar.activation(ke, kraw, AF.Exp)
                nc.scalar.activation(kr2, kraw, AF.Relu)
                nc.vector.scalar_tensor_tensor(
                    out=kf[:, :, u, :], in0=kev, scalar=1.0, in1=kr2v,
                    op0=ALU.min, op1=ALU.add)
                # vm[:,:,u,0:D] = V * mask (bf16), col D = mask
                nc.gpsimd.tensor_tensor(
                    out=vm[:, :, u, 0:D], in0=vrv,
                    in1=mtile[:, p, :].unsqueeze(-1).to_broadcast([128, R, D]),
                    op=ALU.mult)
                nc.gpsimd.tensor_copy(out=vm[:, :, u, D], in_=mtile[:, p, :])

                if u == 1:
                    if prev is not None:
                        tail(*prev)
                        prev = None
                    # KV+Ksum accumulation: [128,130]-wide, 32 steps
                    for r in range(R):
                        nc.tensor.matmul(kv, lhsT=kf[:, r], rhs=vm[:, r],
                                         start=(r == 0), stop=(r == R - 1))
                    # Qf^T batched 4-wide
                    for b in range(R // 4):
                        tp = tp_ps.tile([128, 4, 128], BF16)
                        for j in range(4):
                            nc.tensor.transpose(tp[:, j], qf[:, 4 * b + j],
                                                identity)
                        nc.scalar.activation(
                            qt[:, 4 * b:4 * b + 4, :], tp, AF.Copy)

                    prev = (g, kv, qt)
            tail(*prev, last=True)
    nc.finalize()
    return nc


_NC_CACHE = None


def _get_nc():
    global _NC_CACHE
    if _NC_CACHE is None:
        _NC_CACHE = build_bass()
    return _NC_CACHE


def kernel(Q: np.ndarray, K: np.ndarray, V: np.ndarray, mask: np.ndarray,
           _trace: bool = False):
    B, H = 4, 16
    NP = B * H
    per = NP // N_CORES
    Qr = np.ascontiguousarray(np.asarray(Q, dtype=np.float32).reshape(NP, S, D))
    Kr = np.ascontiguousarray(np.asarray(K, dtype=np.float32).reshape(NP, S, D))
    Vr = np.ascontiguousarray(np.asarray(V, dtype=np.float32).reshape(NP, S, D))
    Mr = np.ascontiguousarray(np.asarray(mask, dtype=np.float32).reshape(NP, S))

    in_maps = []
    for i in range(N_CORES):
        sl = slice(i * per, (i + 1) * per)
        in_maps.append({
            "Q": np.ascontiguousarray(Qr[sl]),
            "K": np.ascontiguousarray(Kr[sl]),
            "V": np.ascontiguousarray(Vr[sl]),
            "mask": np.ascontiguousarray(Mr[sl]),
        })

    nc = _get_nc()
    res = run_bass_kernel_spmd(nc, in_maps, core_ids=list(range(N_CORES)),
                               trace=_trace)
    out = np.concatenate([r["O"] for r in res.results], axis=0)
    if _trace:
        kernel._last_results = res
    return out.reshape(B, H, S, D)
